# revision 15
# baseline (speedup 1.0000x reference)
"""Causal dense self-attention (B=2, T=2048, C=2048, 16 heads, D=128) on 8
Trainium2 NeuronCores.

Sharding: core = b*4 + hg  (b = batch, hg = head-group of 4 heads).
Per core:
  qkv:  full x^T resident in SBUF via 64 hardware DMA-transposes (bf16),
        qT/kT per head in [d, t] layout, v in [t, d] layout; weights
        streamed once.
  attn: S^T tiles [t2:128 x t1:512] = kT.T @ qT, exp on ACT (no max
        subtraction -- scores are O(5)), causal mask via gpsimd
        affine_select, PV and broadcast row-sums (ones-matmul) accumulated
        on PE, normalize = ACT evict + DVE divide.
  proj: AllGather y^T column blocks (4 x 0.5MB per group of 4 cores), then
        out^T[c_out, t] = w_proj.T @ y^T_full for this core's 512 c_out cols.
Host reassembles: out[b][:, hg*512:(hg+1)*512] = outT.T.

Matmul operands are bf16 (1 cyc/row on PE; fp32 PSUM accumulate).
"""

import contextlib
import sys

sys.path.insert(0, "/opt/trn_rl_repo")

import ml_dtypes
import numpy as np

import concourse.bacc as bacc
import concourse.mybir as mybir
import concourse.tile as tile
from concourse.bass_utils import run_bass_kernel_spmd

f32 = mybir.dt.float32
bf16 = mybir.dt.bfloat16

T = 2048
C = 2048
N_HEAD_CORE = 4  # heads per core
D = 128
JW = N_HEAD_CORE * D  # 512: per-core slice width of q/k/v and c_out
TC = 512  # t1-group width in attention / proj
ATTN_MULT = 1.0 / np.sqrt(D)
N_CORES = 8
GROUPS = [[0, 1, 2, 3], [4, 5, 6, 7]]

_CACHED = {}


def build_nc():
    nc = bacc.Bacc("TRN2", target_bir_lowering=False, debug=False)
    dt = bf16

    x = nc.dram_tensor("x", [T, C], dt, kind="ExternalInput")
    wq = nc.dram_tensor("wq", [C, JW], dt, kind="ExternalInput")
    wk = nc.dram_tensor("wk", [C, JW], dt, kind="ExternalInput")
    wv = nc.dram_tensor("wv", [C, JW], dt, kind="ExternalInput")
    wp = nc.dram_tensor("wp", [C, JW], dt, kind="ExternalInput")
    bq = nc.dram_tensor("bq", [JW], f32, kind="ExternalInput")
    bk = nc.dram_tensor("bk", [JW], f32, kind="ExternalInput")
    bv = nc.dram_tensor("bv", [JW], dt, kind="ExternalInput")
    bp = nc.dram_tensor("bp", [JW], f32, kind="ExternalInput")
    ones_d = nc.dram_tensor("ones", [128, 128], dt, kind="ExternalInput")
    outT = nc.dram_tensor("outT", [JW, T], f32, kind="ExternalOutput")

    # DRAM bounce buffers for the per-(t-chunk, head) AllGather of y^T
    yt_in = [
        [nc.dram_tensor(f"yt_in_{g}_{h}", [128, TC], dt) for h in range(4)]
        for g in range(4)
    ]
    yt_out = [
        [nc.dram_tensor(f"yt_out_{g}_{h}", [4 * 128, TC], dt) for h in range(4)]
        for g in range(4)
    ]

    n_cc = C // 128  # 16 contraction chunks
    n_tt = T // 128  # 16 t tiles

    with tile.TileContext(nc) as tc:
        with contextlib.ExitStack() as ctx:
            const_pool = ctx.enter_context(tc.tile_pool(name="const", bufs=1))
            qkv_pool = ctx.enter_context(tc.tile_pool(name="qkv", bufs=1))

            # ---- constants ----
            ones128 = const_pool.tile([128, 128], dt, name="ones128")
            nc.sync.dma_start(out=ones128[:], in_=ones_d.ap())
            ones_row = const_pool.tile([1, 128], dt, name="ones_row")
            nc.sync.dma_start(out=ones_row[:], in_=ones_d.ap()[0:1, :])

            bq_sb = const_pool.tile([128, 4], f32, name="bq_sb")
            bk_sb = const_pool.tile([128, 4], f32, name="bk_sb")
            bp_sb = const_pool.tile([128, 4], f32, name="bp_sb")
            nc.sync.dma_start(
                out=bq_sb[:], in_=bq.ap().rearrange("(j p) -> p j", p=128)
            )
            nc.sync.dma_start(
                out=bk_sb[:], in_=bk.ap().rearrange("(j p) -> p j", p=128)
            )
            nc.sync.dma_start(
                out=bp_sb[:], in_=bp.ap().rearrange("(j p) -> p j", p=128)
            )
            bv_sb = const_pool.tile([1, JW], dt, name="bv_sb")
            nc.sync.dma_start(out=bv_sb[:], in_=bv.ap()[None, :])

            # ---- resident qkv outputs ----
            qT = [
                qkv_pool.tile([128, T], dt, name=f"qT_{h}")
                for h in range(N_HEAD_CORE)
            ]
            kT = [
                qkv_pool.tile([128, T], dt, name=f"kT_{h}")
                for h in range(N_HEAD_CORE)
            ]
            v_sb = [
                qkv_pool.tile([128, JW], dt, name=f"v_{ti}") for ti in range(n_tt)
            ]

            # ================= phase 1: qkv =================
            with (
                tc.tile_pool(name="xt", bufs=1) as xt_pool,
                tc.tile_pool(name="w", bufs=8) as w_pool,
                tc.tile_pool(name="wv", bufs=1) as wv_pool,
                tc.tile_pool(name="qkv_psum", bufs=3, space="PSUM") as qkv_psum,
                tc.tile_pool(name="v_psum", bufs=2, space="PSUM") as v_psum,
            ):
                # full x^T resident as 64 tiles [128, TC]; transposes issued
                # tq-major so the first qkv chains unblock after 16 DMAs
                xT = [
                    [
                        xt_pool.tile([128, TC], dt, name=f"xT_{ci}_{tq}")
                        for tq in range(T // TC)
                    ]
                    for ci in range(n_cc)
                ]
                for tq in range(T // TC):
                    for ci in range(n_cc):
                        nc.sync.dma_start(
                            out=xT[ci][tq][:],
                            in_=x.ap()[
                                tq * TC : (tq + 1) * TC, ci * 128 : (ci + 1) * 128
                            ],
                            transpose=True,
                        )

                # wv resident: [128, JW] per ci (loaded once)
                wv_t = [
                    wv_pool.tile([128, JW], dt, name=f"wv_{ci}")
                    for ci in range(n_cc)
                ]
                for ci in range(0, n_cc, 4):
                    for cj in range(4):
                        nc.scalar.dma_start(
                            out=wv_t[ci + cj][:],
                            in_=wv.ap()[(ci + cj) * 128 : (ci + cj + 1) * 128, :],
                        )

                # 1b: qT / kT weights resident: [128, 16*128] per j-tile
                wj_t = []
                for jj in range(2 * N_HEAD_CORE):
                    h = jj % N_HEAD_CORE
                    w_dram = wq if jj < N_HEAD_CORE else wk
                    wj = w_pool.tile([128, n_cc * 128], dt, tag="w", name="wj")
                    nc.scalar.dma_start(
                        out=wj[:].rearrange("p (c j) -> p c j", c=n_cc),
                        in_=w_dram.ap()[:, h * 128 : (h + 1) * 128].rearrange(
                            "(c p) j -> p c j", p=128
                        ),
                    )
                    wj_t.append(wj)

                # tq outer so PE work tracks the transpose stream
                for tq in range(T // TC):
                    for jj in range(2 * N_HEAD_CORE):
                        h = jj % N_HEAD_CORE
                        is_q = jj < N_HEAD_CORE
                        ps = qkv_psum.tile([128, TC], f32, tag="qkv")
                        for ci in range(n_cc):
                            nc.tensor.matmul(
                                ps[:],
                                wj_t[jj][:, ci * 128 : (ci + 1) * 128],
                                xT[ci][tq][:],
                                start=(ci == 0),
                                stop=(ci == n_cc - 1),
                            )
                        dst = qT[h] if is_q else kT[h]
                        bias = bq_sb if is_q else bk_sb
                        nc.scalar.activation(
                            dst[:, tq * TC : (tq + 1) * TC],
                            ps[:],
                            mybir.ActivationFunctionType.Identity,
                            bias=bias[:, h : h + 1],
                        )
                    # v for this tq's 4 t-tiles
                    for ti in range(tq * (TC // 128), (tq + 1) * (TC // 128)):
                        ti_in = ti % (TC // 128)
                        ps = v_psum.tile([128, JW], f32, tag="v")
                        for ci in range(n_cc):
                            nc.tensor.matmul(
                                ps[:],
                                xT[ci][tq][:, ti_in * 128 : (ti_in + 1) * 128],
                                wv_t[ci][:],
                                start=(ci == 0),
                                stop=False,
                            )
                        nc.tensor.matmul(
                            ps[:], ones_row[:], bv_sb[:], start=False, stop=True
                        )
                        nc.scalar.copy(v_sb[ti][:], ps[:])

            # ============ phase 2+3 interleaved per t1-group g ============
            with (
                tc.tile_pool(name="wp_pool", bufs=1) as wp_pool,
                tc.tile_pool(name="p", bufs=18) as p_pool,
                tc.tile_pool(name="r", bufs=2) as r_pool,
                tc.tile_pool(name="y", bufs=2) as y_pool,
                tc.tile_pool(name="ytl", bufs=18) as ytl_pool,
                tc.tile_pool(name="o", bufs=2) as o_pool,
                tc.tile_pool(name="s_psum", bufs=2, space="PSUM") as s_psum,
                tc.tile_pool(name="acc_psum", bufs=2, space="PSUM") as acc_psum,
                tc.tile_pool(name="o_psum", bufs=2, space="PSUM") as o_psum,
            ):
                wp_sb = wp_pool.tile([128, n_cc * JW], dt, name="wp_sb")
                for ci in range(n_cc):
                    nc.scalar.dma_start(
                        out=wp_sb[:, ci * JW : (ci + 1) * JW],
                        in_=wp.ap()[ci * 128 : (ci + 1) * 128, :],
                    )

                def attn_group(g):
                    jmax = 4 * g + 4
                    for h in range(N_HEAD_CORE):
                        # pass 1: S^T -> exp -> mask, all j (pT tiles kept)
                        pts = []
                        for j in range(jmax):
                            ps_s = s_psum.tile([128, TC], f32, tag="s", name="ps_s")
                            nc.tensor.matmul(
                                ps_s[:],
                                kT[h][:, j * 128 : (j + 1) * 128],
                                qT[h][:, g * TC : (g + 1) * TC],
                                start=True,
                                stop=True,
                            )
                            pT = p_pool.tile([128, TC], dt, tag="p", name="pT")
                            nc.scalar.activation(
                                pT[:],
                                ps_s[:],
                                mybir.ActivationFunctionType.Exp,
                                scale=float(ATTN_MULT),
                            )
                            r = j - 4 * g
                            if r >= 0:
                                # causal: keep iff f - p - 128*r >= 0
                                nc.gpsimd.affine_select(
                                    out=pT[:],
                                    in_=pT[:],
                                    compare_op=mybir.AluOpType.is_ge,
                                    fill=0.0,
                                    base=-128 * r,
                                    pattern=[[1, TC]],
                                    channel_multiplier=-1,
                                )
                            pts.append(pT)
                        # pass 2: PV + rowsum accumulation
                        ps_y = acc_psum.tile([128, TC], f32, tag="ps_y", name="ps_y")
                        ps_r = acc_psum.tile([128, TC], f32, tag="ps_r", name="ps_r")
                        for j in range(jmax):
                            nc.tensor.matmul(
                                ps_y[:],
                                v_sb[j][:, h * 128 : (h + 1) * 128],
                                pts[j][:],
                                start=(j == 0),
                                stop=(j == jmax - 1),
                            )
                            nc.tensor.matmul(
                                ps_r[:],
                                ones128[:],
                                pts[j][:],
                                start=(j == 0),
                                stop=(j == jmax - 1),
                            )
                        recip = r_pool.tile([128, TC], f32, tag="recip", name="recip")
                        nc.vector.reciprocal(recip[:], ps_r[:])
                        yt_sb = y_pool.tile([128, TC], dt, tag="yt", name="yt_sb")
                        nc.vector.tensor_mul(yt_sb[:], ps_y[:], recip[:])
                        nc.scalar.dma_start(
                            out=yt_in[g][h].ap(), in_=yt_sb[:]
                        )
                        nc.gpsimd.collective_compute(
                            "AllGather",
                            mybir.AluOpType.bypass,
                            replica_groups=GROUPS,
                            ins=[yt_in[g][h].ap()],
                            outs=[yt_out[g][h].ap()],
                        )

                def proj_group(g):
                    # proj: out^T[c_out, t] = wp.T @ yT_full
                    ytiles = []
                    for ci in range(n_cc):
                        rank, h = divmod(ci, 4)
                        ytile = ytl_pool.tile([128, TC], dt, tag="ytl", name="ytile")
                        nc.sync.dma_start(
                            out=ytile[:],
                            in_=yt_out[g][h].ap()[
                                rank * 128 : (rank + 1) * 128, :
                            ],
                        )
                        ytiles.append(ytile)
                    for co in range(JW // 128):
                        ps_o = o_psum.tile([128, TC], f32, tag="o", name="ps_o")
                        for ci in range(n_cc):
                            nc.tensor.matmul(
                                ps_o[:],
                                wp_sb[
                                    :, ci * JW + co * 128 : ci * JW + (co + 1) * 128
                                ],
                                ytiles[ci][:],
                                start=(ci == 0),
                                stop=(ci == n_cc - 1),
                            )
                        o_sb = o_pool.tile([128, TC], f32, tag="o_sb", name="o_sb")
                        nc.scalar.activation(
                            o_sb[:],
                            ps_o[:],
                            mybir.ActivationFunctionType.Identity,
                            bias=bp_sb[:, co : co + 1],
                        )
                        nc.sync.dma_start(
                            out=outT.ap()[
                                co * 128 : (co + 1) * 128, g * TC : (g + 1) * TC
                            ],
                            in_=o_sb[:],
                        )

                # software pipeline: attn(g+1) is emitted before proj(g) so the
                # PE stream has work while AllGather(g) is in flight
                attn_group(0)
                for g in range(1, 4):
                    attn_group(g)
                    proj_group(g - 1)
                proj_group(3)

    nc.compile()
    return nc


def kernel(x, w_qkv, b_qkv, w_proj, b_proj, _trace=False):
    x = np.ascontiguousarray(np.asarray(x, dtype=np.float32))
    w_qkv = np.ascontiguousarray(np.asarray(w_qkv, dtype=np.float32))
    b_qkv = np.ascontiguousarray(np.asarray(b_qkv, dtype=np.float32))
    w_proj = np.ascontiguousarray(np.asarray(w_proj, dtype=np.float32))
    b_proj = np.ascontiguousarray(np.asarray(b_proj, dtype=np.float32))
    B = x.shape[0]

    if "nc" not in _CACHED:
        _CACHED["nc"] = build_nc()
    nc = _CACHED["nc"]

    np_dt = ml_dtypes.bfloat16

    def cvt(a):
        return np.ascontiguousarray(a.astype(np_dt))

    in_maps = []
    for core in range(N_CORES):
        b, hg = divmod(core, 4)
        s = slice(hg * JW, (hg + 1) * JW)
        in_maps.append(
            {
                "x": cvt(x[b]),
                "wq": cvt(w_qkv[:, 0:C][:, s]),
                "wk": cvt(w_qkv[:, C : 2 * C][:, s]),
                "wv": cvt(w_qkv[:, 2 * C : 3 * C][:, s]),
                "wp": cvt(w_proj[:, s]),
                "bq": np.ascontiguousarray(b_qkv[0:C][s]),
                "bk": np.ascontiguousarray(b_qkv[C : 2 * C][s]),
                "bv": cvt(b_qkv[2 * C : 3 * C][s]),
                "bp": np.ascontiguousarray(b_proj[s]),
                "ones": np.ones((128, 128), dtype=np_dt),
            }
        )

    res = run_bass_kernel_spmd(nc, in_maps, list(range(N_CORES)), trace=_trace)
    _CACHED["last_result"] = res

    out = np.empty((B, T, C), dtype=np.float32)
    for core in range(N_CORES):
        b, hg = divmod(core, 4)
        out[b][:, hg * JW : (hg + 1) * JW] = res.results[core]["outT"].T
    return out
